# revision 1
# baseline (speedup 1.0000x reference)
"""Trainium2 Bass kernel for nn_AttentionEvaluatorModel (batch-data-parallel, 8 cores).

Model (per batch b):
  q = mapper(query, Wq, bq); f = mapper(features, Wf, bf); v = mapper(values, Wv, bv)
  attn = sigmoid(q @ f.T);  w = attn * ftw * mask
  pooled = w @ v;  h = mapper(pooled, Wc, bc);  out = h @ Wout + bout
where mapper layer: x = relu(x @ W + b) + x  ==  max(x @ (W + I) + b, x).

Sharding: pure DP over batch (B=32 -> 4 batches/core, 8 cores, no collectives).

Per-core dataflow (all matmul activations transposed: E on partitions):
  - features/values are cast-loaded f32->bf16 (SWDGE) in natural layout, then
    transposed on-chip via the DMA xbar (dma_start_transpose) into [E, Fi] strips.
  - mapper layers: stationary W' = W+I with a fused max(psum+b, x) epilogue on
    DVE, or plain W with relu(psum+b) on ACT + add on GpSimd/DVE; the three
    forms are cycled to balance engine load.
  - attention is computed in natural [fi, q] layout (f2T chunks stationary),
    sigmoid on ACT straight off PSUM, w = sigmoid * (ftw*mask) on GpSimd.
  - v2 is xbar-transposed back to natural and used as stationary for the
    pooled contraction (PSUM-accumulated over feature chunks).
  - tiny c-mapper + output head finish on-chip; final f32 [4,16,8] DMA'd out.
"""

from contextlib import ExitStack

import numpy as np

import concourse.bass as bass
import concourse.mybir as mybir
import concourse.tile as tile
from concourse import bacc
from concourse.masks import make_identity

B, Q, F, E, NL, L = 32, 16, 4096, 256, 8, 2
NCORES = 8
BPC = B // NCORES          # batches per core = 4
RQ = BPC * Q               # rows for q/c mappers = 64
P = 128
EH = E // P                # e-halves = 2
OC = F // P                # 32  (fi = 32*k + o, k in [0,128), o in [0,32))
RC = 8                     # fi column-chunks per batch for mapper (512 wide each)

F32 = mybir.dt.float32
BF16 = mybir.dt.bfloat16

AF = mybir.ActivationFunctionType
ALU = mybir.AluOpType

MATS = ("q", "f", "v", "c")


def build_nc(repeats=1):
    """Build the per-core Bass graph (same graph for all 8 cores, SPMD)."""
    nc = bacc.Bacc("TRN2", target_bir_lowering=False, debug=False,
                   num_devices=NCORES)

    d_query = nc.dram_tensor("query", [BPC, Q, E], F32, kind="ExternalInput").ap()
    d_feat = nc.dram_tensor("features", [BPC, F, E], F32, kind="ExternalInput").ap()
    d_vals = nc.dram_tensor("values", [BPC, F, E], F32, kind="ExternalInput").ap()
    d_mask = nc.dram_tensor("attention_mask", [BPC, F], F32, kind="ExternalInput").ap()
    d_ftw = nc.dram_tensor("feature_time_weights", [BPC, F], F32, kind="ExternalInput").ap()
    d_W = {m: nc.dram_tensor(f"W{m}", [L, E, E], F32, kind="ExternalInput").ap()
           for m in MATS}
    d_b = {m: nc.dram_tensor(f"b{m}", [L, E], F32, kind="ExternalInput").ap()
           for m in MATS}
    d_Wout = nc.dram_tensor("Wout", [E, NL], F32, kind="ExternalInput").ap()
    d_bout = nc.dram_tensor("bout", [NL], F32, kind="ExternalInput").ap()
    d_out = nc.dram_tensor("out", [BPC, Q, NL], F32, kind="ExternalOutput").ap()

    with tile.TileContext(nc) as tc:
        with ExitStack() as ctx:
            _emit(ctx, tc, nc, d_query, d_feat, d_vals, d_mask, d_ftw,
                  d_W, d_b, d_Wout, d_bout, d_out, repeats=repeats)

    nc.compile()
    return nc


def _emit(ctx, tc, nc, d_query, d_feat, d_vals, d_mask, d_ftw,
          d_W, d_b, d_Wout, d_bout, d_out, repeats=1):
    consts = ctx.enter_context(tc.tile_pool(name="consts", bufs=1))
    stage = ctx.enter_context(tc.tile_pool(name="stage", bufs=2))
    xbuf = ctx.enter_context(tc.tile_pool(name="xbuf", bufs=1))
    small = ctx.enter_context(tc.tile_pool(name="small", bufs=2))
    zps = ctx.enter_context(tc.tile_pool(name="zps", bufs=6, space="PSUM"))
    pps = ctx.enter_context(tc.tile_pool(name="ppsum", bufs=2, space="PSUM"))

    # ---------------- input prefetch machinery ----------------
    def load_and_transpose(d_src, b, kind):
        """Cast-load [F, E] f32 -> bf16 natural, then xbar to .T strips.
        Split into o-halves so the mapper can start on the first half."""
        nat = stage.tile([P, OC, E], BF16, tag="x0n", bufs=3, name="x0n")
        src = d_src[b].rearrange("(p o) e -> p o e", p=P)
        xt = xbuf.tile([P, 2 * OC, P], BF16, tag=f"{kind}xT", bufs=1,
                       name=f"{kind}xT")
        hoc = OC // 2
        for half in range(2):
            osl = slice(half * hoc, (half + 1) * hoc)
            nc.gpsimd.dma_start(nat[:, osl, :], src[:, osl, :])
            nc.sync.dma_start_transpose(
                xt[:, half * OC:(half + 1) * OC, :],
                nat[:, osl, :].rearrange("p o e -> p (o e)"))
        return xt

    loaded = {}

    def prefetch(b):
        """Emit batch b's loads (SWDGE cast + xbar + s tiles) early."""
        if b >= BPC or b in loaded:
            return
        fxt = load_and_transpose(d_feat, b, "f")
        vxt = load_and_transpose(d_vals, b, "v")
        ftw_t = small.tile([P, OC], F32, tag="ftw", name="ftw")
        nc.sync.dma_start(ftw_t[:], d_ftw[b].rearrange("(k o) -> k o", k=P))
        msk_t = small.tile([P, OC], F32, tag="msk", name="msk")
        nc.sync.dma_start(msk_t[:], d_mask[b].rearrange("(k o) -> k o", k=P))
        s_t = small.tile([P, OC], BF16, tag="s", name="s")
        nc.vector.tensor_tensor(s_t[:], ftw_t[:], msk_t[:], ALU.mult)
        loaded[b] = (s_t, fxt, vxt)

    # ---------------- constants / weights ----------------
    ident = consts.tile([P, P], F32, tag="ident")
    make_identity(nc, ident[:])

    # Two bf16 stationary copies per (mat, layer, ei-half), lhsT [ei, eo]:
    #   Wn = W      (for relu(z+b)+x epilogues on ACT+GpSimd/DVE)
    #   Wb = W + I  (for max(z'+b, x) fused epilogues on DVE)
    Wb, Wn = {}, {}
    for m in MATS:
        for l in range(L):
            for ei in range(EH):
                wf = stage.tile([P, E], F32, tag="wstage", name="wstage")
                nc.sync.dma_start(wf[:], d_W[m][l, ei * P:(ei + 1) * P, :])
                wn = consts.tile([P, E], BF16, tag=f"Wn{m}{l}{ei}",
                                 name=f"Wn{m}{l}{ei}")
                nc.vector.tensor_copy(wn[:], wf[:])
                Wn[(m, l, ei)] = wn
                nc.vector.tensor_tensor(wf[:, ei * P:(ei + 1) * P],
                                        wf[:, ei * P:(ei + 1) * P],
                                        ident[:], ALU.add)
                wb = consts.tile([P, E], BF16, tag=f"W{m}{l}{ei}",
                                 name=f"W{m}{l}{ei}")
                nc.vector.tensor_copy(wb[:], wf[:])
                Wb[(m, l, ei)] = wb

    # biases: one contiguous [16, 256] staging load (rows = mat*2+layer), cast
    # to bf16 and xbar-transposed to [128(e_lo), EH(e_hi), 16(row)].
    ball_f = small.tile([16, E], F32, tag="ball_f")
    nc.vector.memset(ball_f[:], 0.0)
    for mi, m in enumerate(MATS):
        nc.sync.dma_start(ball_f[2 * mi:2 * mi + 2, :], d_b[m])
    ball_b = small.tile([16, E], BF16, tag="ball_b")
    nc.vector.tensor_copy(ball_b[:], ball_f[:])
    biasT = consts.tile([P, EH, 16], BF16, tag="biasT")
    nc.sync.dma_start_transpose(biasT[:], ball_b[:])
    bias = {}
    for mi, m in enumerate(MATS):
        for l in range(L):
            bias[(m, l)] = biasT[:, :, 2 * mi + l]

    # batch-0 big loads: after the (small) weight/bias loads in queue order
    prefetch(0)

    # ---------------- epilogue helper ----------------
    # Form A (DVE, uses W+I psum):   x_out = max(psum + b, x_in)   [1 op]
    # Form B (ACT+GpSimd, plain W):  x_out = relu(psum + b) + x_in [2 ops]
    # Form C (ACT+DVE, plain W):     same as B with the add on DVE
    ep_count = [0]
    FORM_CYCLE = "AAABBCCC"

    def next_form():
        u = ep_count[0]
        ep_count[0] += 1
        return FORM_CYCLE[u % len(FORM_CYCLE)]

    def epilogue(form, zpsum, b_ap, x_in, x_out):
        if form == "A":
            nc.vector.scalar_tensor_tensor(
                out=x_out, in0=zpsum, scalar=b_ap, in1=x_in,
                op0=ALU.add, op1=ALU.max)
        else:
            t = small.tile(list(zpsum.shape), BF16, tag="eptmp", name="eptmp")
            nc.scalar.activation(t[:], zpsum, AF.Relu, bias=b_ap)
            if form == "B":
                nc.gpsimd.tensor_tensor(x_out, t[:], x_in, ALU.add)
            else:
                nc.vector.tensor_tensor(x_out, t[:], x_in, ALU.add)

    # ---------------- q-mapper (tiny) ----------------
    q0f = small.tile([RQ, E], F32, tag="q0f")
    nc.sync.dma_start(q0f[:], d_query.rearrange("b q e -> (b q) e"))
    q0b = small.tile([RQ, E], BF16, tag="q0b")
    nc.vector.tensor_copy(q0b[:], q0f[:])
    qT = consts.tile([P, EH, RQ], BF16, tag="qT0")
    nc.sync.dma_start_transpose(qT[:], q0b[:])

    def small_mapper(xT, mat):
        """xT [128, EH, RQ] bf16 -> mapper output, same layout."""
        cur = xT
        for l in range(L):
            nxt = consts.tile([P, EH, RQ], BF16, tag=f"{mat}T{l + 1}",
                              name=f"{mat}T{l + 1}")
            for eo in range(EH):
                ps = zps.tile([P, RQ], F32, tag="zpsum", name="qcpsum")
                for ei in range(EH):
                    nc.tensor.matmul(ps[:], lhsT=Wb[(mat, l, ei)][:, eo * P:(eo + 1) * P],
                                     rhs=cur[:, ei, :],
                                     start=(ei == 0), stop=(ei == EH - 1))
                epilogue("A", ps[:], bias[(mat, l)][:, eo:eo + 1], cur[:, eo, :],
                         nxt[:, eo, :])
            cur = nxt
        return cur

    q2T = small_mapper(qT, "q")

    # ---------------- per-batch pipeline ----------------
    # fi enumeration per batch: fi = 32*k + o  (k in [0,128), o in [0,32))
    # xbar-entry tile X [128(p'), 64(j=2o+h), 128(k)]: X[p', 2o+h, k] =
    #   x.T[e = 128h + p', fi = 32k + o]
    pooledT = consts.tile([P, EH, RQ], BF16, tag="pooledT")

    def strip(x_interleaved, h):
        """e-half strip [128, OC, 128] view of the interleaved xbar-entry tile."""
        return x_interleaved[:, h::2, :]

    def big_mapper(xt_il, mat, b):
        """Mapper on interleaved entry tile; outputs per-half tiles
        [128, OC, 128] (o-major)."""
        cur = None  # list of per-half APs
        for l in range(L):
            nxt = [xbuf.tile([P, OC, P], BF16, tag=f"{mat}T{l + 1}h{h}", bufs=1,
                             name=f"{mat}T{l + 1}h{h}") for h in range(EH)]
            for rc in range(RC):
                o0 = 4 * rc
                for eo in range(EH):
                    form = next_form()
                    Wsta = Wb if form == "A" else Wn
                    ps = zps.tile([P, 4, P], F32, tag="zpsum", name="zpsum")
                    for ei in range(EH):
                        if cur is None:
                            rhs = strip(xt_il, ei)[:, o0:o0 + 4, :]
                        else:
                            rhs = cur[ei][:, o0:o0 + 4, :]
                        nc.tensor.matmul(ps[:], lhsT=Wsta[(mat, l, ei)][:, eo * P:(eo + 1) * P],
                                         rhs=rhs, start=(ei == 0), stop=(ei == EH - 1))
                    if cur is None:
                        res = strip(xt_il, eo)[:, o0:o0 + 4, :]
                    else:
                        res = cur[eo][:, o0:o0 + 4, :]
                    epilogue(form, ps[:], bias[(mat, l)][:, eo:eo + 1], res,
                             nxt[eo][:, o0:o0 + 4, :])
            cur = nxt
        return cur

    for rep in range(repeats):
      if rep > 0:
          loaded.clear()
          prefetch(0)
      for b in range(BPC):
        s_t, fxt, vxt = loaded.pop(b)

        # ---- f path
        f2 = big_mapper(fxt, "f", b)

        # ---- next batch's loads overlap this batch's second half
        prefetch(b + 1)

        # ---- v path
        v2 = big_mapper(vxt, "v", b)

        # ---- attention logits -> sigmoid -> w
        aps_t = zps.tile([P, OC, Q], F32, tag="zpsum", name="attnps")
        for o in range(OC):
            for h in range(EH):
                nc.tensor.matmul(aps_t[:, o, :], lhsT=f2[h][:, o, :],
                                 rhs=q2T[:, h, b * Q:(b + 1) * Q],
                                 start=(h == 0), stop=(h == EH - 1))
        att_b = small.tile([P, OC, Q], BF16, tag="attnsb", name="attnsb")
        nc.scalar.activation(att_b[:], aps_t[:], AF.Sigmoid)
        w_t = small.tile([P, OC, Q], BF16, tag="w", name="w")
        nc.gpsimd.tensor_tensor(w_t[:], att_b[:],
                                s_t[:, :, None].to_broadcast((P, OC, Q)),
                                ALU.mult)

        # ---- v2 back to natural layout: v2n_h [128(p'), OC(j), 128(k')] =
        #      v2[fi = 32p' + j, e = 128h + k']
        v2n = []
        hoc = OC // 2
        for h in range(EH):
            vn = xbuf.tile([P, OC, P], BF16, tag=f"v2nh{h}", bufs=1,
                           name=f"v2nh{h}")
            for half in range(2):
                osl = slice(half * hoc, (half + 1) * hoc)
                nc.sync.dma_start_transpose(
                    vn[:, osl, :],
                    v2[h][:, osl, :].rearrange("p o e -> p (o e)"))
            v2n.append(vn)

        # ---- pooled.T accumulation: [e_h, q] per half, drained per batch
        for h in range(EH):
            pp = pps.tile([P, Q], F32, tag="poolps", name="poolps")
            for j in range(OC):
                nc.tensor.matmul(pp[:],
                                 lhsT=v2n[h][:, j, :], rhs=w_t[:, j, :],
                                 start=(j == 0), stop=(j == OC - 1))
            nc.vector.tensor_copy(pooledT[:, h, b * Q:(b + 1) * Q], pp[:])

    # ---------------- head constants (off the startup critical path) --------
    with nc.allow_non_contiguous_dma(reason="tiny one-time wout load"):
        woutf = small.tile([P, EH, NL], F32, tag="woutf")
        nc.sync.dma_start(woutf[:], d_Wout.rearrange("(h p) n -> p h n", p=P))
    woutb = consts.tile([P, EH, NL], BF16, tag="woutb")
    nc.vector.tensor_copy(woutb[:], woutf[:])
    boutf = small.tile([1, NL], F32, tag="boutf")
    nc.sync.dma_start(boutf[:], d_bout.rearrange("(a n) -> a n", a=1))
    boutb = consts.tile([1, NL], BF16, tag="boutb")
    nc.vector.tensor_copy(boutb[:], boutf[:])
    ones_row = consts.tile([1, RQ], BF16, tag="ones_row")
    nc.vector.memset(ones_row[:], 1.0)

    # ---------------- pooled -> c-mapper -> head ----------------
    h2T = small_mapper(pooledT, "c")

    out_ps = zps.tile([RQ, NL], F32, tag="zpsum", name="outps")
    for h in range(EH):
        nc.tensor.matmul(out_ps[:], lhsT=h2T[:, h, :], rhs=woutb[:, h, :],
                         start=(h == 0), stop=False)
    nc.tensor.matmul(out_ps[:], lhsT=ones_row[:], rhs=boutb[:],
                     start=False, stop=True)
    out_sb = small.tile([RQ, NL], F32, tag="outsb")
    nc.vector.tensor_copy(out_sb[:], out_ps[:])
    nc.sync.dma_start(d_out.rearrange("b q n -> (b q) n"), out_sb[:])


def make_in_maps(inputs):
    """Shard the full inputs into 8 per-core input maps (pure batch slicing)."""
    in_maps = []
    for c in range(NCORES):
        sl = slice(c * BPC, (c + 1) * BPC)
        m = {
            "query": inputs["query"][sl],
            "features": inputs["features"][sl],
            "values": inputs["values"][sl],
            "attention_mask": inputs["attention_mask"][sl],
            "feature_time_weights": inputs["feature_time_weights"][sl],
            "Wq": inputs["Wq"], "bq": inputs["bq"],
            "Wf": inputs["Wf"], "bf": inputs["bf"],
            "Wv": inputs["Wv"], "bv": inputs["bv"],
            "Wc": inputs["Wc"], "bc": inputs["bc"],
            "Wout": inputs["Wout"], "bout": inputs["bout"],
        }
        in_maps.append({k: np.ascontiguousarray(v, dtype=np.float32)
                        for k, v in m.items()})
    return in_maps


_NC_CACHE = {}


def get_nc():
    if "nc" not in _NC_CACHE:
        _NC_CACHE["nc"] = build_nc()
    return _NC_CACHE["nc"]


def kernel(**inputs) -> np.ndarray:
    from concourse.bass_utils import run_bass_kernel_spmd

    inputs = {k: np.asarray(v) for k, v in inputs.items()}
    nc = get_nc()
    in_maps = make_in_maps(inputs)
    res = run_bass_kernel_spmd(nc, in_maps, core_ids=list(range(NCORES)))
    out = np.concatenate([res.results[c]["out"] for c in range(NCORES)], axis=0)
    return out.astype(np.float32)

